# revision 3
# baseline (speedup 1.0000x reference)
"""CoAttention module kernel for Trainium2 (8 NeuronCores).

Problem: B=4 pairs of (left, right) feature maps [B, C=2048, H=W=48].
Two attention directions per pair -> 8 independent attention problems,
one per core (data parallel, no cross-core communication).

Per core (qf = query features [C, HW], rf = reference features [C, HW]):
    Q = Wq @ qf + bq          [HC=256, HW=2304]   (fp32r matmuls)
    K = Wk @ rf + bk          [HC=256, HW=2304]
    S = Q^T K                 [2304, 2304]        (fp32r, streamed by
                                                   128-row i-tiles)
    P = softmax(S, axis=-1)                       (exact row max, ACT exp)
    O = V P^T, V = rf         [C, HW]             (bf16 matmuls)

Host side: shards 8 (batch, direction) problems over 8 cores, runs the
SPMD NEFF, and concatenates [orig, weighted] channel-wise.
"""

import sys

sys.path.insert(0, "/opt/trn_rl_repo")

import numpy as np

import concourse.bass as bass
import concourse.mybir as mybir
import concourse.tile as tile
from concourse import bacc
from concourse.bass_utils import run_bass_kernel_spmd
from concourse.masks import make_identity

B, C, H, W = 4, 2048, 48, 48
HW = H * W  # 2304
HC = 256

F32 = mybir.dt.float32
F32R = mybir.dt.float32r
BF16 = mybir.dt.bfloat16

NCC = C // 128  # 16 channel chunks
NHC = HC // 128  # 2 head-channel halves
NJT = HW // 128  # 18 j tiles
NIT = HW // 128  # 18 i tiles
JH = 768  # phase-1 j stripe (fits 2 PSUM banks, chunks 512+256)
NJH = HW // JH  # 3
S_CHUNKS = [(0, 512), (512, 512), (1024, 512), (1536, 512), (2048, 256)]
SUPERS = [(0, 512), (512, 512), (1024, 512), (1536, 512), (2048, 256)]

_CACHED_NC = None


def build_nc():
    nc = bacc.Bacc("TRN2", target_bir_lowering=False, debug=False, num_devices=8)

    qf = nc.dram_tensor("qf", [C, HW], F32, kind="ExternalInput").ap()
    rf = nc.dram_tensor("rf", [C, HW], F32, kind="ExternalInput").ap()
    Wq = nc.dram_tensor("Wq", [HC, C], F32, kind="ExternalInput").ap()
    bq = nc.dram_tensor("bq", [HC], F32, kind="ExternalInput").ap()
    Wk = nc.dram_tensor("Wk", [HC, C], F32, kind="ExternalInput").ap()
    bk = nc.dram_tensor("bk", [HC], F32, kind="ExternalInput").ap()
    out = nc.dram_tensor("out", [C, HW], F32, kind="ExternalOutput").ap()

    with tile.TileContext(nc) as tc:
        build_tile_kernel(tc, out, qf, rf, Wq, bq, Wk, bk)

    nc.compile()
    return nc


def build_tile_kernel(tc, out, qf, rf, Wq, bq, Wk, bk):
    nc = tc.nc

    with (
        tc.tile_pool(name="persist", bufs=1) as persist,
        tc.tile_pool(name="consts", bufs=1) as consts,
    ):
        # Persistent tensors (live across phases).
        # VT[jp, jc, c] = rf[c, jc*128 + jp] in bf16   (72 KB/part)
        VT = persist.tile([128, NJT, C], BF16, tag="VT")
        # Q_sb[hp, h, i], K_sb[hp, h, j] in f32r       (18 KB/part each)
        Q_sb = persist.tile([128, NHC, HW], F32R, tag="Q")
        K_sb = persist.tile([128, NHC, HW], F32R, tag="K")

        ident_f = consts.tile([128, 128], F32, tag="idf")
        ident_bf = consts.tile([128, 128], BF16, tag="idbf")
        make_identity(nc, ident_f[:])
        make_identity(nc, ident_bf[:])
        bq_t = consts.tile([128, NHC], F32, tag="bq")
        bk_t = consts.tile([128, NHC], F32, tag="bk")
        nc.sync.dma_start(out=bq_t[:], in_=bq.rearrange("(h p) -> p h", p=128))
        nc.sync.dma_start(out=bk_t[:], in_=bk.rearrange("(h p) -> p h", p=128))

        # ---- Phase 0: transpose weights:  WT[cp, cc, hc] = W[hc, cc*128+cp]
        with tc.tile_pool(name="wt", bufs=1) as wt_pool:
            WqT = wt_pool.tile([128, NCC, HC], F32R, tag="WqT")
            WkT = wt_pool.tile([128, NCC, HC], F32R, tag="WkT")
            with (
                tc.tile_pool(name="wraw", bufs=1) as wraw_pool,
                tc.tile_pool(name="wtpsum", bufs=4, space="PSUM") as wt_psum,
            ):
                for name, Wsrc, WT in (("q", Wq, WqT), ("k", Wk, WkT)):
                    for h in range(NHC):
                        wr = wraw_pool.tile([128, C], F32, tag="wraw")
                        nc.sync.dma_start(
                            out=wr[:], in_=Wsrc[h * 128 : (h + 1) * 128, :]
                        )
                        for cc in range(NCC):
                            pt = wt_psum.tile([128, 128], F32, tag="wtp")
                            nc.tensor.transpose(
                                pt[:], wr[:, cc * 128 : (cc + 1) * 128], ident_f[:]
                            )
                            nc.vector.tensor_copy(
                                WT[:, cc, h * 128 : (h + 1) * 128], pt[:]
                            )

            # ---- Phase 1: projections Q/K + VT build.
            with (
                tc.tile_pool(name="stream", bufs=3) as stream,
                tc.tile_pool(name="streamr", bufs=2) as streamr,
                tc.tile_pool(name="projpsum", bufs=1, space="PSUM") as proj_psum,
                tc.tile_pool(name="trpsum", bufs=2, space="PSUM") as tr_psum,
            ):
                for is_k in (False, True):
                    src = rf if is_k else qf
                    WT = WkT if is_k else WqT
                    dst = K_sb if is_k else Q_sb
                    bias = bk_t if is_k else bq_t
                    for jh in range(NJH):
                        j0 = jh * JH
                        psums = []
                        for h in range(NHC):
                            pp = proj_psum.tile(
                                [128, JH], F32, tag=f"proj{h}", name=f"psum_proj{h}"
                            )
                            psums.append(pp)
                        for cc in range(NCC):
                            xt = stream.tile([128, JH], F32, tag="xt")
                            nc.sync.dma_start(
                                out=xt[:],
                                in_=src[cc * 128 : (cc + 1) * 128, j0 : j0 + JH],
                            )
                            xr = streamr.tile([128, JH], F32R, tag="xr")
                            nc.vector.tensor_copy(xr[:], xt[:])
                            for h in range(NHC):
                                for n0, nn in ((0, 512), (512, 256)):
                                    nc.tensor.matmul(
                                        psums[h][:, n0 : n0 + nn],
                                        WT[:, cc, h * 128 : (h + 1) * 128],
                                        xr[:, n0 : n0 + nn],
                                        start=(cc == 0),
                                        stop=(cc == NCC - 1),
                                    )
                            if is_k:
                                # VT tiles from this rf chunk
                                xbf = streamr.tile([128, JH], BF16, tag="xbf")
                                nc.vector.tensor_copy(xbf[:], xt[:])
                                for jl in range(JH // 128):
                                    jc = (j0 + jl * 128) // 128
                                    ptb = tr_psum.tile([128, 128], BF16, tag="vtp")
                                    nc.tensor.transpose(
                                        ptb[:],
                                        xbf[:, jl * 128 : (jl + 1) * 128],
                                        ident_bf[:],
                                    )
                                    nc.scalar.copy(
                                        VT[:, jc, cc * 128 : (cc + 1) * 128], ptb[:]
                                    )
                        for h in range(NHC):
                            nc.scalar.activation(
                                dst[:, h, j0 : j0 + JH],
                                psums[h][:],
                                mybir.ActivationFunctionType.Identity,
                                bias=bias[:, h : h + 1],
                                scale=1.0,
                            )

        # ---- Phase 2+3: attention per i-super-tile.
        with (
            tc.tile_pool(name="attn", bufs=2) as attn,
            tc.tile_pool(name="pt", bufs=1) as pt_pool,
            tc.tile_pool(name="spsum", bufs=2, space="PSUM") as s_psum,
            tc.tile_pool(name="ptpsum", bufs=2, space="PSUM") as p_psum,
            tc.tile_pool(name="opsum", bufs=2, space="PSUM") as o_psum,
            tc.tile_pool(name="small", bufs=3) as small,
        ):
            for sup_off, sup_len in SUPERS:
                PT = pt_pool.tile([128, NJT, 512], BF16, tag="PT")
                for it in range(sup_off // 128, (sup_off + sup_len) // 128):
                    i0 = it * 128
                    S_sb = attn.tile([128, HW], F32, tag="S")
                    for j0, jn in S_CHUNKS:
                        ps = s_psum.tile([128, 512], F32, tag="S")
                        for h in range(NHC):
                            nc.tensor.matmul(
                                ps[:, :jn],
                                Q_sb[:, h, i0 : i0 + 128],
                                K_sb[:, h, j0 : j0 + jn],
                                start=(h == 0),
                                stop=(h == NHC - 1),
                            )
                        nc.vector.tensor_copy(S_sb[:, j0 : j0 + jn], ps[:, :jn])
                    negmax = small.tile([128, 1], F32, tag="negmax")
                    nc.vector.tensor_reduce(
                        negmax[:],
                        S_sb[:],
                        axis=mybir.AxisListType.X,
                        op=mybir.AluOpType.max,
                        negate=True,
                    )
                    P_bf = attn.tile([128, HW], BF16, tag="P")
                    sumexp = small.tile([128, 1], F32, tag="sumexp")
                    nc.scalar.activation(
                        P_bf[:],
                        S_sb[:],
                        mybir.ActivationFunctionType.Exp,
                        bias=negmax[:],
                        scale=1.0,
                        accum_out=sumexp[:],
                    )
                    rcp = small.tile([128, 1], F32, tag="rcp")
                    nc.vector.reciprocal(rcp[:], sumexp[:])
                    nc.vector.tensor_scalar_mul(P_bf[:], P_bf[:], rcp[:])
                    # transpose P into PT[:, jc, it_local*128 : ...]
                    il = i0 - sup_off
                    for jc in range(NJT):
                        ptb = p_psum.tile([128, 128], BF16, tag="ptp")
                        nc.tensor.transpose(
                            ptb[:], P_bf[:, jc * 128 : (jc + 1) * 128], ident_bf[:]
                        )
                        nc.scalar.copy(PT[:, jc, il : il + 128], ptb[:])
                # AV for this super-tile
                for cc in range(NCC):
                    po = o_psum.tile([128, 512], F32, tag="O")
                    for jc in range(NJT):
                        nc.tensor.matmul(
                            po[:, :sup_len],
                            VT[:, jc, cc * 128 : (cc + 1) * 128],
                            PT[:, jc, :sup_len],
                            start=(jc == 0),
                            stop=(jc == NJT - 1),
                        )
                    O_sb = attn.tile([128, 512], F32, tag="O")
                    nc.vector.tensor_copy(O_sb[:, :sup_len], po[:, :sup_len])
                    nc.sync.dma_start(
                        out=out[cc * 128 : (cc + 1) * 128, sup_off : sup_off + sup_len],
                        in_=O_sb[:, :sup_len],
                    )


def get_nc():
    global _CACHED_NC
    if _CACHED_NC is None:
        _CACHED_NC = build_nc()
    return _CACHED_NC


def kernel(left_features, right_features, Wq, bq, Wk, bk):
    left = np.ascontiguousarray(np.asarray(left_features, dtype=np.float32)).reshape(
        B, C, HW
    )
    right = np.ascontiguousarray(np.asarray(right_features, dtype=np.float32)).reshape(
        B, C, HW
    )
    Wq = np.ascontiguousarray(np.asarray(Wq, dtype=np.float32))
    Wk = np.ascontiguousarray(np.asarray(Wk, dtype=np.float32))
    bq = np.ascontiguousarray(np.asarray(bq, dtype=np.float32))
    bk = np.ascontiguousarray(np.asarray(bk, dtype=np.float32))

    nc = get_nc()

    # cores 0..3: weighted_r for batch b (query=left, ref=right)
    # cores 4..7: weighted_l for batch b (query=right, ref=left)
    in_maps = []
    for b in range(B):
        in_maps.append(
            {"qf": left[b], "rf": right[b], "Wq": Wq, "bq": bq, "Wk": Wk, "bk": bk}
        )
    for b in range(B):
        in_maps.append(
            {"qf": right[b], "rf": left[b], "Wq": Wq, "bq": bq, "Wk": Wk, "bk": bk}
        )

    res = run_bass_kernel_spmd(nc, in_maps, core_ids=list(range(8)))

    weighted_r = np.stack([res.results[b]["out"] for b in range(B)]).reshape(
        B, C, H, W
    )
    weighted_l = np.stack([res.results[B + b]["out"] for b in range(B)]).reshape(
        B, C, H, W
    )
    left4 = left.reshape(B, C, H, W)
    right4 = right.reshape(B, C, H, W)
    left_attended = np.concatenate([left4, weighted_l], axis=1)
    right_attended = np.concatenate([right4, weighted_r], axis=1)
    return (left_attended, right_attended)


# revision 13
# speedup vs baseline: 44.0114x; 44.0114x over previous
"""CoAttention module kernel for Trainium2 (8 NeuronCores).

Problem: B=4 pairs of (left, right) feature maps [B, C=2048, H=W=48].
Two attention directions per pair -> 8 independent attention problems,
one per core (data parallel, no cross-core communication).

Per core (qf = query features [C, HW], rf = reference features [C, HW]):
    Q = Wq @ qf + bq          [HC=256, HW=2304]   (fp32r matmuls)
    K = Wk @ rf + bk          [HC=256, HW=2304]
    S = Q^T K                 [2304, 2304]        (fp32r, by 128-row i-tiles)
    P = softmax(S, axis=-1)                       (exact row max, ACT exp)
    O = V P^T, V = rf         [C, HW]             (bf16 matmuls)

Schedule (software-pipelined by emission order):
  Region A (DMA-bound): K and Q projection stripes interleaved; rf chunks
    are also cast to bf16 and PE-transposed into the resident VT.
    W^T tiles are staged in DRAM (built once by PE) and streamed back per
    stripe to keep SBUF small.
  Region B (PE-bound): attention. The S/softmax/P-transpose work for
    i-super-tile n+1 is interleaved into the AV matmul stream of
    super-tile n so the PE never idles on the softmax dependency chain
    (DVE max -> ACT exp -> DVE normalize).

Host side: shards 8 (batch, direction) problems over 8 cores, runs the
SPMD NEFF, and concatenates [orig, weighted] channel-wise.
"""

import sys

sys.path.insert(0, "/opt/trn_rl_repo")

import numpy as np

import concourse.bass as bass
import concourse.mybir as mybir
import concourse.tile as tile
from concourse import bacc
from concourse.bass_utils import run_bass_kernel_spmd
from concourse.masks import make_identity

B, C, H, W = 4, 2048, 48, 48
HW = H * W  # 2304
HC = 256

F32 = mybir.dt.float32
F32R = mybir.dt.float32r
BF16 = mybir.dt.bfloat16

NCC = C // 128  # 16 channel chunks
NHC = HC // 128  # 2 head-channel halves
NJT = HW // 128  # 18 j tiles
NIT = HW // 128  # 18 i tiles
# Projection stripes (PSUM tile [128, 2, w] must fit in 4 banks).
STRIPES = [(0, 1024), (1024, 1024), (2048, 256)]
# S chunks / AV i-super-tiles (one PSUM bank each).
SUPERS = [(0, 512), (512, 512), (1024, 512), (1536, 512), (2048, 256)]
S_CHUNKS = SUPERS

_CACHED_NC = None


def build_nc():
    nc = bacc.Bacc("TRN2", target_bir_lowering=False, debug=False, num_devices=8)

    qf = nc.dram_tensor("qf", [C, HW], F32, kind="ExternalInput").ap()
    rf = nc.dram_tensor("rf", [C, HW], F32, kind="ExternalInput").ap()
    Wq = nc.dram_tensor("Wq", [HC, C], F32, kind="ExternalInput").ap()
    bq = nc.dram_tensor("bq", [HC], F32, kind="ExternalInput").ap()
    Wk = nc.dram_tensor("Wk", [HC, C], F32, kind="ExternalInput").ap()
    bk = nc.dram_tensor("bk", [HC], F32, kind="ExternalInput").ap()
    out = nc.dram_tensor("out", [C, HW], F32, kind="ExternalOutput").ap()

    with tile.TileContext(nc) as tc:
        build_tile_kernel(tc, out, qf, rf, Wq, bq, Wk, bk)

    nc.compile()
    return nc


def build_tile_kernel(tc, out, qf, rf, Wq, bq, Wk, bk):
    nc = tc.nc

    with (
        tc.tile_pool(name="persist", bufs=1) as persist,
        tc.tile_pool(name="consts", bufs=1) as consts,
        tc.tile_pool(name="dram", bufs=1, space="DRAM") as dram_pool,
    ):
        # Persistent tensors (live across phases).
        VT = persist.tile([128, NJT, C], BF16, tag="VT")  # VT[jp, jc, c]
        Q_sb = persist.tile([128, NHC, HW], F32R, tag="Q")  # [hp, h, i]
        K_sb = persist.tile([128, NHC, HW], F32R, tag="K")  # [hp, h, j]

        ident_f = consts.tile([128, 128], F32, tag="idf")
        ident_bf = consts.tile([128, 128], BF16, tag="idbf")
        make_identity(nc, ident_f[:])
        make_identity(nc, ident_bf[:])
        bq_t = consts.tile([128, NHC], F32, tag="bq")
        bk_t = consts.tile([128, NHC], F32, tag="bk")
        nc.sync.dma_start(out=bq_t[:], in_=bq.rearrange("(h p) -> p h", p=128))
        nc.sync.dma_start(out=bk_t[:], in_=bk.rearrange("(h p) -> p h", p=128))

        # ---- Phase 0 + Region A under the W^T pool (SBUF-resident there).
        with tc.tile_pool(name="wt", bufs=1) as wt_pool:
            WqT = wt_pool.tile([128, NCC, HC], F32R, tag="WqT")
            WkT = wt_pool.tile([128, NCC, HC], F32R, tag="WkT")
            with (
                tc.tile_pool(name="wraw", bufs=2) as wraw_pool,
                tc.tile_pool(name="wtpsum", bufs=4, space="PSUM") as wt_psum,
            ):
                for Wsrc, WT in ((Wq, WqT), (Wk, WkT)):
                    for h in range(NHC):
                        wr = wraw_pool.tile([128, C], F32, tag="wraw")
                        nc.sync.dma_start(
                            out=wr[:], in_=Wsrc[h * 128 : (h + 1) * 128, :]
                        )
                        for cc in range(NCC):
                            pt = wt_psum.tile([128, 128], F32, tag="wtp")
                            nc.tensor.transpose(
                                pt[:], wr[:, cc * 128 : (cc + 1) * 128], ident_f[:]
                            )
                            nc.vector.tensor_copy(
                                WT[:, cc, h * 128 : (h + 1) * 128], pt[:]
                            )

            # ---- Region A: projections (K and Q interleaved) + VT build.
            with (
                tc.tile_pool(name="streamx", bufs=4) as streamx,
                tc.tile_pool(name="streamr", bufs=3) as streamr,
                tc.tile_pool(name="streambf", bufs=3) as streambf,
                tc.tile_pool(name="projpsum", bufs=1, space="PSUM") as proj_psum,
                tc.tile_pool(name="trpsum", bufs=3, space="PSUM") as tr_psum,
            ):

                def proj_stripe(is_k, s):
                    src = rf if is_k else qf
                    WT = WkT if is_k else WqT
                    dst = K_sb if is_k else Q_sb
                    bias = bk_t if is_k else bq_t
                    j0, jw = STRIPES[s]
                    # one PSUM tile per h half: tiles are bank-padded, so the
                    # two halves never share a bank (bank-clearing on
                    # start=True would corrupt a shared bank's accumulation)
                    pp = []
                    for h in range(NHC):
                        pph = proj_psum.tile(
                            [128, jw], F32, tag=f"proj{h}", name=f"pproj_{is_k}_{s}_{h}"
                        )
                        pp.append(pph)
                    nck = 512  # matmul N chunk (one PSUM bank)
                    for cc in range(NCC):
                        xt = streamx.tile(
                            [128, jw], F32, tag="xt", name=f"xt{is_k}{s}{cc}"
                        )
                        nc.sync.dma_start(
                            out=xt[:],
                            in_=src[cc * 128 : (cc + 1) * 128, j0 : j0 + jw],
                        )
                        xr = streamr.tile(
                            [128, jw], F32R, tag="xr", name=f"xr{is_k}{s}{cc}"
                        )
                        nc.vector.tensor_copy(xr[:], xt[:])
                        for h in range(NHC):
                            for n0 in range(0, jw, nck):
                                nn = min(nck, jw - n0)
                                nc.tensor.matmul(
                                    pp[h][:, n0 : n0 + nn],
                                    WT[:, cc, h * 128 : (h + 1) * 128],
                                    xr[:, n0 : n0 + nn],
                                    start=(cc == 0),
                                    stop=(cc == NCC - 1),
                                )
                        if is_k:
                            xbf = streambf.tile(
                                [128, jw], BF16, tag="xbf", name=f"xbf{s}{cc}"
                            )
                            nc.vector.tensor_copy(xbf[:], xt[:])
                            # transpose 128x128 blocks in groups of 4 sharing
                            # one PSUM bank, evict with a single strided copy
                            gw = 4
                            for g0 in range(0, jw // 128, gw):
                                gn = min(gw, jw // 128 - g0)
                                ptb = tr_psum.tile(
                                    [128, gw * 128],
                                    BF16,
                                    tag="vtp",
                                    name=f"vtp{s}{cc}{g0}",
                                )
                                for jl in range(g0, g0 + gn):
                                    # slices share one PSUM bank: only the
                                    # first write may clear it (start=True)
                                    nc.tensor.matmul(
                                        ptb[:, (jl - g0) * 128 : (jl - g0 + 1) * 128],
                                        xbf[:, jl * 128 : (jl + 1) * 128],
                                        ident_bf[:],
                                        is_transpose=True,
                                        start=(jl == g0),
                                        stop=(jl == g0 + gn - 1),
                                        skip_group_check=True,
                                    )
                                jc0 = j0 // 128 + g0
                                dst_ap = VT[
                                    :, jc0 : jc0 + gn, cc * 128 : (cc + 1) * 128
                                ]
                                src_ap = ptb[:, : gn * 128].rearrange(
                                    "p (g b) -> p g b", g=gn
                                )
                                if cc % 2 == 0:
                                    nc.scalar.copy(dst_ap, src_ap)
                                else:
                                    nc.vector.tensor_copy(dst_ap, src_ap)
                    for h in range(NHC):
                        nc.scalar.activation(
                            dst[:, h, j0 : j0 + jw],
                            pp[h][:],
                            mybir.ActivationFunctionType.Identity,
                            bias=bias[:, h : h + 1],
                            scale=1.0,
                        )

                for s in range(len(STRIPES)):
                    proj_stripe(True, s)
                    proj_stripe(False, s)

        # ---- Region B: attention, software-pipelined across super-tiles.
        with (
            tc.tile_pool(name="sbuf_s", bufs=2) as pool_s,
            tc.tile_pool(name="sbuf_p", bufs=2) as pool_p,
            tc.tile_pool(name="sbuf_o", bufs=3) as pool_o,
            tc.tile_pool(name="pt", bufs=2) as pt_pool,
            tc.tile_pool(name="small", bufs=4) as small,
            tc.tile_pool(name="spsum", bufs=3, space="PSUM") as s_psum,
            tc.tile_pool(name="ptpsum", bufs=2, space="PSUM") as p_psum,
            tc.tile_pool(name="opsum", bufs=3, space="PSUM") as o_psum,
        ):
            PTs = {}

            def super_of(it):
                for n, (off, ln) in enumerate(SUPERS):
                    if off <= it * 128 < off + ln:
                        return n
                raise AssertionError(it)

            def s_work(it):
                i0 = it * 128
                S_sb = pool_s.tile([128, HW], F32, tag="S", name=f"S_{it}")
                for j0, jn in S_CHUNKS:
                    ps = s_psum.tile([128, 512], F32, tag="S", name=f"psS_{it}_{j0}")
                    for h in range(NHC):
                        nc.tensor.matmul(
                            ps[:, :jn],
                            Q_sb[:, h, i0 : i0 + 128],
                            K_sb[:, h, j0 : j0 + jn],
                            start=(h == 0),
                            stop=(h == NHC - 1),
                        )
                    nc.vector.tensor_copy(S_sb[:, j0 : j0 + jn], ps[:, :jn])
                negmax = small.tile([128, 1], F32, tag="negmax", name=f"nm_{it}")
                nc.vector.tensor_reduce(
                    negmax[:],
                    S_sb[:],
                    axis=mybir.AxisListType.X,
                    op=mybir.AluOpType.max,
                    negate=True,
                )
                P_bf = pool_p.tile([128, HW], BF16, tag="P", name=f"P_{it}")
                sumexp = small.tile([128, 1], F32, tag="sumexp", name=f"se_{it}")
                nc.scalar.activation(
                    P_bf[:],
                    S_sb[:],
                    mybir.ActivationFunctionType.Exp,
                    bias=negmax[:],
                    scale=1.0,
                    accum_out=sumexp[:],
                )
                rcp = small.tile([128, 1], F32, tag="rcp", name=f"rcp_{it}")
                nc.vector.reciprocal(rcp[:], sumexp[:])
                nc.vector.tensor_scalar_mul(P_bf[:], P_bf[:], rcp[:])
                return P_bf

            def tr_work(it, P_bf):
                n = super_of(it)
                PT = PTs[n]
                il = it * 128 - SUPERS[n][0]
                for jc in range(NJT):
                    ptb = p_psum.tile(
                        [128, 128], BF16, tag="ptp", name=f"ptp_{it}_{jc}"
                    )
                    nc.tensor.transpose(
                        ptb[:], P_bf[:, jc * 128 : (jc + 1) * 128], ident_bf[:]
                    )
                    nc.scalar.copy(PT[:, jc, il : il + 128], ptb[:])

            def av_chunk(n, cc):
                sup_off, sup_len = SUPERS[n]
                PT = PTs[n]
                po = o_psum.tile([128, 512], F32, tag="O", name=f"psO_{n}_{cc}")
                for jc in range(NJT):
                    nc.tensor.matmul(
                        po[:, :sup_len],
                        VT[:, jc, cc * 128 : (cc + 1) * 128],
                        PT[:, jc, :sup_len],
                        start=(jc == 0),
                        stop=(jc == NJT - 1),
                    )
                O_sb = pool_o.tile([128, 512], F32, tag="O", name=f"O_{n}_{cc}")
                nc.vector.tensor_copy(O_sb[:, :sup_len], po[:, :sup_len])
                nc.sync.dma_start(
                    out=out[cc * 128 : (cc + 1) * 128, sup_off : sup_off + sup_len],
                    in_=O_sb[:, :sup_len],
                )

            def tiles_of(n):
                off, ln = SUPERS[n]
                return list(range(off // 128, (off + ln) // 128))

            def alloc_pt(n):
                PTs[n] = pt_pool.tile([128, NJT, 512], BF16, tag="PT", name=f"PT_{n}")

            # Prologue: build super 0's PT (pipelined at tile granularity).
            alloc_pt(0)
            pending_P = {}
            t0 = tiles_of(0)
            pending_P[t0[0]] = s_work(t0[0])
            for idx in range(1, len(t0) + 1):
                if idx < len(t0):
                    pending_P[t0[idx]] = s_work(t0[idx])
                done = t0[idx - 1]
                tr_work(done, pending_P.pop(done))

            for n in range(len(SUPERS)):
                sched = {}
                if n + 1 < len(SUPERS):
                    alloc_pt(n + 1)
                    nxt = tiles_of(n + 1)
                    for k, t in enumerate(nxt):
                        sched.setdefault(1 + 2 * k, []).append(("S", t))
                    for k, t in enumerate(nxt):
                        sched.setdefault(8 + 2 * k, []).append(("TR", t))
                for cc in range(NCC):
                    av_chunk(n, cc)
                    for kind, t in sched.get(cc, []):
                        if kind == "S":
                            pending_P[t] = s_work(t)
                        else:
                            tr_work(t, pending_P.pop(t))


def get_nc():
    global _CACHED_NC
    if _CACHED_NC is None:
        _CACHED_NC = build_nc()
    return _CACHED_NC


def kernel(left_features, right_features, Wq, bq, Wk, bk):
    left = np.ascontiguousarray(np.asarray(left_features, dtype=np.float32)).reshape(
        B, C, HW
    )
    right = np.ascontiguousarray(np.asarray(right_features, dtype=np.float32)).reshape(
        B, C, HW
    )
    Wq = np.ascontiguousarray(np.asarray(Wq, dtype=np.float32))
    Wk = np.ascontiguousarray(np.asarray(Wk, dtype=np.float32))
    bq = np.ascontiguousarray(np.asarray(bq, dtype=np.float32))
    bk = np.ascontiguousarray(np.asarray(bk, dtype=np.float32))

    nc = get_nc()

    # cores 0..3: weighted_r for batch b (query=left, ref=right)
    # cores 4..7: weighted_l for batch b (query=right, ref=left)
    in_maps = []
    for b in range(B):
        in_maps.append(
            {"qf": left[b], "rf": right[b], "Wq": Wq, "bq": bq, "Wk": Wk, "bk": bk}
        )
    for b in range(B):
        in_maps.append(
            {"qf": right[b], "rf": left[b], "Wq": Wq, "bq": bq, "Wk": Wk, "bk": bk}
        )

    res = run_bass_kernel_spmd(nc, in_maps, core_ids=list(range(8)))

    weighted_r = np.stack([res.results[b]["out"] for b in range(B)]).reshape(B, C, H, W)
    weighted_l = np.stack([res.results[B + b]["out"] for b in range(B)]).reshape(
        B, C, H, W
    )
    left4 = left.reshape(B, C, H, W)
    right4 = right.reshape(B, C, H, W)
    left_attended = np.concatenate([left4, weighted_l], axis=1)
    right_attended = np.concatenate([right4, weighted_r], axis=1)
    return (left_attended, right_attended)


# revision 15
# speedup vs baseline: 150.4538x; 3.4185x over previous
"""CoAttention module kernel for Trainium2 (8 NeuronCores).

Problem: B=4 pairs of (left, right) feature maps [B, C=2048, H=W=48].
Two attention directions per pair -> 8 independent attention problems,
one per core (data parallel, no cross-core communication).

Per core (qf = query features [C, HW], rf = reference features [C, HW]):
    Q = Wq @ qf + bq          [HC=256, HW=2304]   (fp32r matmuls)
    K = Wk @ rf + bk          [HC=256, HW=2304]
    S = Q^T K                 [2304, 2304]        (fp32r, by 128-row i-tiles)
    P = softmax(S, axis=-1)                       (exact row max, ACT exp)
    O = V P^T, V = rf         [C, HW]             (bf16 matmuls)

Schedule (software-pipelined by emission order):
  Region A (DMA-bound): K and Q projection stripes interleaved; rf chunks
    are also cast to bf16 and PE-transposed into the resident VT.
    W^T tiles are staged in DRAM (built once by PE) and streamed back per
    stripe to keep SBUF small.
  Region B (PE-bound): attention. The S/softmax/P-transpose work for
    i-super-tile n+1 is interleaved into the AV matmul stream of
    super-tile n so the PE never idles on the softmax dependency chain
    (DVE max -> ACT exp -> DVE normalize).

Host side: shards 8 (batch, direction) problems over 8 cores, runs the
SPMD NEFF, and concatenates [orig, weighted] channel-wise.
"""

import sys

sys.path.insert(0, "/opt/trn_rl_repo")

import numpy as np

import concourse.bass as bass
import concourse.mybir as mybir
import concourse.tile as tile
from concourse import bacc
from concourse.bass_utils import run_bass_kernel_spmd
from concourse.masks import make_identity

B, C, H, W = 4, 2048, 48, 48
HW = H * W  # 2304
HC = 256

F32 = mybir.dt.float32
F32R = mybir.dt.float32r
BF16 = mybir.dt.bfloat16

NCC = C // 128  # 16 channel chunks
NHC = HC // 128  # 2 head-channel halves
NJT = HW // 128  # 18 j tiles
NIT = HW // 128  # 18 i tiles
# Projection stripes (PSUM tile [128, 2, w] must fit in 4 banks).
STRIPES = [(0, 1024), (1024, 1024), (2048, 256)]
# S chunks / AV i-super-tiles (one PSUM bank each).
SUPERS = [(0, 512), (512, 512), (1024, 512), (1536, 512), (2048, 256)]
S_CHUNKS = SUPERS

_CACHED_NC = None


def build_nc(reps=1):
    nc = bacc.Bacc("TRN2", target_bir_lowering=False, debug=False, num_devices=8)

    qf = nc.dram_tensor("qf", [C, HW], F32, kind="ExternalInput").ap()
    rf = nc.dram_tensor("rf", [C, HW], F32, kind="ExternalInput").ap()
    Wq = nc.dram_tensor("Wq", [HC, C], F32, kind="ExternalInput").ap()
    bq = nc.dram_tensor("bq", [HC], F32, kind="ExternalInput").ap()
    Wk = nc.dram_tensor("Wk", [HC, C], F32, kind="ExternalInput").ap()
    bk = nc.dram_tensor("bk", [HC], F32, kind="ExternalInput").ap()
    out = nc.dram_tensor("out", [C, HW], F32, kind="ExternalOutput").ap()

    with tile.TileContext(nc) as tc:
        for _ in range(reps):
            build_tile_kernel(tc, out, qf, rf, Wq, bq, Wk, bk)

    nc.compile()
    return nc


def build_tile_kernel(tc, out, qf, rf, Wq, bq, Wk, bk):
    nc = tc.nc

    with (
        tc.tile_pool(name="persist", bufs=1) as persist,
        tc.tile_pool(name="consts", bufs=1) as consts,
        tc.tile_pool(name="dram", bufs=1, space="DRAM") as dram_pool,
    ):
        # Persistent tensors (live across phases).
        VT = persist.tile([128, NJT, C], BF16, tag="VT")  # VT[jp, jc, c]
        Q_sb = persist.tile([128, NHC, HW], F32R, tag="Q")  # [hp, h, i]
        K_sb = persist.tile([128, NHC, HW], F32R, tag="K")  # [hp, h, j]

        ident_f = consts.tile([128, 128], F32, tag="idf")
        ident_bf = consts.tile([128, 128], BF16, tag="idbf")
        make_identity(nc, ident_f[:])
        make_identity(nc, ident_bf[:])
        bq_t = consts.tile([128, NHC], F32, tag="bq")
        bk_t = consts.tile([128, NHC], F32, tag="bk")
        nc.sync.dma_start(out=bq_t[:], in_=bq.rearrange("(h p) -> p h", p=128))
        nc.sync.dma_start(out=bk_t[:], in_=bk.rearrange("(h p) -> p h", p=128))

        # ---- Phase 0 + Region A under the W^T pool (SBUF-resident there).
        with tc.tile_pool(name="wt", bufs=1) as wt_pool:
            WqT = wt_pool.tile([128, NCC, HC], F32R, tag="WqT")
            WkT = wt_pool.tile([128, NCC, HC], F32R, tag="WkT")
            with (
                tc.tile_pool(name="wraw", bufs=2) as wraw_pool,
                tc.tile_pool(name="wtpsum", bufs=4, space="PSUM") as wt_psum,
            ):
                for Wsrc, WT in ((Wq, WqT), (Wk, WkT)):
                    for h in range(NHC):
                        wr = wraw_pool.tile([128, C], F32, tag="wraw")
                        nc.sync.dma_start(
                            out=wr[:], in_=Wsrc[h * 128 : (h + 1) * 128, :]
                        )
                        for cc in range(NCC):
                            pt = wt_psum.tile([128, 128], F32, tag="wtp")
                            nc.tensor.transpose(
                                pt[:], wr[:, cc * 128 : (cc + 1) * 128], ident_f[:]
                            )
                            nc.vector.tensor_copy(
                                WT[:, cc, h * 128 : (h + 1) * 128], pt[:]
                            )

            # ---- Region A: projections (K and Q interleaved) + VT build.
            with (
                tc.tile_pool(name="streamx", bufs=6) as streamx,
                tc.tile_pool(name="streamr", bufs=3) as streamr,
                tc.tile_pool(name="streambf", bufs=3) as streambf,
                tc.tile_pool(name="projpsum", bufs=1, space="PSUM") as proj_psum,
                tc.tile_pool(name="trpsum", bufs=3, space="PSUM") as tr_psum,
            ):

                def proj_stripe(is_k, s):
                    src = rf if is_k else qf
                    WT = WkT if is_k else WqT
                    dst = K_sb if is_k else Q_sb
                    bias = bk_t if is_k else bq_t
                    j0, jw = STRIPES[s]
                    # one PSUM tile per h half: tiles are bank-padded, so the
                    # two halves never share a bank (bank-clearing on
                    # start=True would corrupt a shared bank's accumulation)
                    pp = []
                    for h in range(NHC):
                        pph = proj_psum.tile(
                            [128, jw], F32, tag=f"proj{h}", name=f"pproj_{is_k}_{s}_{h}"
                        )
                        pp.append(pph)
                    nck = 512  # matmul N chunk (one PSUM bank)
                    for cc in range(NCC):
                        xt = streamx.tile(
                            [128, jw], F32, tag="xt", name=f"xt{is_k}{s}{cc}"
                        )
                        nc.sync.dma_start(
                            out=xt[:],
                            in_=src[cc * 128 : (cc + 1) * 128, j0 : j0 + jw],
                        )
                        xr = streamr.tile(
                            [128, jw], F32R, tag="xr", name=f"xr{is_k}{s}{cc}"
                        )
                        nc.vector.tensor_copy(xr[:], xt[:])
                        for h in range(NHC):
                            for n0 in range(0, jw, nck):
                                nn = min(nck, jw - n0)
                                nc.tensor.matmul(
                                    pp[h][:, n0 : n0 + nn],
                                    WT[:, cc, h * 128 : (h + 1) * 128],
                                    xr[:, n0 : n0 + nn],
                                    start=(cc == 0),
                                    stop=(cc == NCC - 1),
                                )
                        if is_k:
                            xbf = streambf.tile(
                                [128, jw], BF16, tag="xbf", name=f"xbf{s}{cc}"
                            )
                            nc.vector.tensor_copy(xbf[:], xt[:])
                            # transpose 128x128 blocks in groups of 4 sharing
                            # one PSUM bank, evict with a single strided copy
                            gw = 4
                            for g0 in range(0, jw // 128, gw):
                                gn = min(gw, jw // 128 - g0)
                                ptb = tr_psum.tile(
                                    [128, gw * 128],
                                    BF16,
                                    tag="vtp",
                                    name=f"vtp{s}{cc}{g0}",
                                )
                                for jl in range(g0, g0 + gn):
                                    # slices share one PSUM bank: only the
                                    # first write may clear it (start=True)
                                    nc.tensor.matmul(
                                        ptb[:, (jl - g0) * 128 : (jl - g0 + 1) * 128],
                                        xbf[:, jl * 128 : (jl + 1) * 128],
                                        ident_bf[:],
                                        is_transpose=True,
                                        start=(jl == g0),
                                        stop=(jl == g0 + gn - 1),
                                        skip_group_check=True,
                                    )
                                jc0 = j0 // 128 + g0
                                dst_ap = VT[
                                    :, jc0 : jc0 + gn, cc * 128 : (cc + 1) * 128
                                ]
                                src_ap = ptb[:, : gn * 128].rearrange(
                                    "p (g b) -> p g b", g=gn
                                )
                                if cc % 2 == 0:
                                    nc.scalar.copy(dst_ap, src_ap)
                                else:
                                    nc.vector.tensor_copy(dst_ap, src_ap)
                    for h in range(NHC):
                        nc.scalar.activation(
                            dst[:, h, j0 : j0 + jw],
                            pp[h][:],
                            mybir.ActivationFunctionType.Identity,
                            bias=bias[:, h : h + 1],
                            scale=1.0,
                        )

                for s in range(len(STRIPES)):
                    proj_stripe(True, s)
                    proj_stripe(False, s)

        # ---- Region B: attention, software-pipelined across super-tiles.
        with (
            tc.tile_pool(name="sbuf_s", bufs=2) as pool_s,
            tc.tile_pool(name="sbuf_p", bufs=2) as pool_p,
            tc.tile_pool(name="sbuf_o", bufs=3) as pool_o,
            tc.tile_pool(name="pt", bufs=2) as pt_pool,
            tc.tile_pool(name="small", bufs=4) as small,
            tc.tile_pool(name="spsum", bufs=3, space="PSUM") as s_psum,
            tc.tile_pool(name="ptpsum", bufs=2, space="PSUM") as p_psum,
            tc.tile_pool(name="opsum", bufs=3, space="PSUM") as o_psum,
        ):
            PTs = {}

            def super_of(it):
                for n, (off, ln) in enumerate(SUPERS):
                    if off <= it * 128 < off + ln:
                        return n
                raise AssertionError(it)

            def s_work(it):
                i0 = it * 128
                S_sb = pool_s.tile([128, HW], F32, tag="S", name=f"S_{it}")
                for j0, jn in S_CHUNKS:
                    ps = s_psum.tile([128, 512], F32, tag="S", name=f"psS_{it}_{j0}")
                    for h in range(NHC):
                        nc.tensor.matmul(
                            ps[:, :jn],
                            Q_sb[:, h, i0 : i0 + 128],
                            K_sb[:, h, j0 : j0 + jn],
                            start=(h == 0),
                            stop=(h == NHC - 1),
                        )
                    nc.vector.tensor_copy(S_sb[:, j0 : j0 + jn], ps[:, :jn])
                negmax = small.tile([128, 1], F32, tag="negmax", name=f"nm_{it}")
                nc.vector.tensor_reduce(
                    negmax[:],
                    S_sb[:],
                    axis=mybir.AxisListType.X,
                    op=mybir.AluOpType.max,
                    negate=True,
                )
                P_bf = pool_p.tile([128, HW], BF16, tag="P", name=f"P_{it}")
                sumexp = small.tile([128, 1], F32, tag="sumexp", name=f"se_{it}")
                nc.scalar.activation(
                    P_bf[:],
                    S_sb[:],
                    mybir.ActivationFunctionType.Exp,
                    bias=negmax[:],
                    scale=1.0,
                    accum_out=sumexp[:],
                )
                rcp = small.tile([128, 1], F32, tag="rcp", name=f"rcp_{it}")
                nc.vector.reciprocal(rcp[:], sumexp[:])
                nc.vector.tensor_scalar_mul(P_bf[:], P_bf[:], rcp[:])
                return P_bf

            def tr_work(it, P_bf):
                n = super_of(it)
                PT = PTs[n]
                il = it * 128 - SUPERS[n][0]
                for jc in range(NJT):
                    ptb = p_psum.tile(
                        [128, 128], BF16, tag="ptp", name=f"ptp_{it}_{jc}"
                    )
                    nc.tensor.transpose(
                        ptb[:], P_bf[:, jc * 128 : (jc + 1) * 128], ident_bf[:]
                    )
                    nc.scalar.copy(PT[:, jc, il : il + 128], ptb[:])

            def av_chunk(n, cc):
                sup_off, sup_len = SUPERS[n]
                PT = PTs[n]
                po = o_psum.tile([128, 512], F32, tag="O", name=f"psO_{n}_{cc}")
                for jc in range(NJT):
                    nc.tensor.matmul(
                        po[:, :sup_len],
                        VT[:, jc, cc * 128 : (cc + 1) * 128],
                        PT[:, jc, :sup_len],
                        start=(jc == 0),
                        stop=(jc == NJT - 1),
                    )
                O_sb = pool_o.tile([128, 512], F32, tag="O", name=f"O_{n}_{cc}")
                nc.vector.tensor_copy(O_sb[:, :sup_len], po[:, :sup_len])
                nc.sync.dma_start(
                    out=out[cc * 128 : (cc + 1) * 128, sup_off : sup_off + sup_len],
                    in_=O_sb[:, :sup_len],
                )

            def tiles_of(n):
                off, ln = SUPERS[n]
                return list(range(off // 128, (off + ln) // 128))

            def alloc_pt(n):
                PTs[n] = pt_pool.tile([128, NJT, 512], BF16, tag="PT", name=f"PT_{n}")

            # Prologue: build super 0's PT (pipelined at tile granularity).
            alloc_pt(0)
            pending_P = {}
            t0 = tiles_of(0)
            pending_P[t0[0]] = s_work(t0[0])
            for idx in range(1, len(t0) + 1):
                if idx < len(t0):
                    pending_P[t0[idx]] = s_work(t0[idx])
                done = t0[idx - 1]
                tr_work(done, pending_P.pop(done))

            for n in range(len(SUPERS)):
                sched = {}
                if n + 1 < len(SUPERS):
                    alloc_pt(n + 1)
                    nxt = tiles_of(n + 1)
                    for k, t in enumerate(nxt):
                        sched.setdefault(1 + 2 * k, []).append(("S", t))
                    for k, t in enumerate(nxt):
                        sched.setdefault(8 + 2 * k, []).append(("TR", t))
                for cc in range(NCC):
                    av_chunk(n, cc)
                    for kind, t in sched.get(cc, []):
                        if kind == "S":
                            pending_P[t] = s_work(t)
                        else:
                            tr_work(t, pending_P.pop(t))


def get_nc():
    global _CACHED_NC
    if _CACHED_NC is None:
        _CACHED_NC = build_nc()
    return _CACHED_NC


def kernel(left_features, right_features, Wq, bq, Wk, bk):
    left = np.ascontiguousarray(np.asarray(left_features, dtype=np.float32)).reshape(
        B, C, HW
    )
    right = np.ascontiguousarray(np.asarray(right_features, dtype=np.float32)).reshape(
        B, C, HW
    )
    Wq = np.ascontiguousarray(np.asarray(Wq, dtype=np.float32))
    Wk = np.ascontiguousarray(np.asarray(Wk, dtype=np.float32))
    bq = np.ascontiguousarray(np.asarray(bq, dtype=np.float32))
    bk = np.ascontiguousarray(np.asarray(bk, dtype=np.float32))

    nc = get_nc()

    # cores 0..3: weighted_r for batch b (query=left, ref=right)
    # cores 4..7: weighted_l for batch b (query=right, ref=left)
    in_maps = []
    for b in range(B):
        in_maps.append(
            {"qf": left[b], "rf": right[b], "Wq": Wq, "bq": bq, "Wk": Wk, "bk": bk}
        )
    for b in range(B):
        in_maps.append(
            {"qf": right[b], "rf": left[b], "Wq": Wq, "bq": bq, "Wk": Wk, "bk": bk}
        )

    res = run_bass_kernel_spmd(nc, in_maps, core_ids=list(range(8)))

    weighted_r = np.stack([res.results[b]["out"] for b in range(B)]).reshape(B, C, H, W)
    weighted_l = np.stack([res.results[B + b]["out"] for b in range(B)]).reshape(
        B, C, H, W
    )
    left4 = left.reshape(B, C, H, W)
    right4 = right.reshape(B, C, H, W)
    left_attended = np.concatenate([left4, weighted_l], axis=1)
    right_attended = np.concatenate([right4, weighted_r], axis=1)
    return (left_attended, right_attended)
